# revision 23
# baseline (speedup 1.0000x reference)
"""Single-head causal attention (LinformerHead) on 8 Trainium2 cores.

Problem: x[4,2048,1024], Wq/Wk/Wv[64,1024] ->
    softmax(causal(x@Wq.T @ (x@Wk.T).T / 8)) @ (x@Wv.T)   [4,2048,64]

Sharding (one SPMD program, 8 cores): each batch's causal triangle(2048)
decomposes into 2 triangles(1024) + 1 rectangle(1024q x 1024k). Core
(b, h) computes triangle of query-half h of batch b, plus half the
rectangle (all 1024 rect queries x 512 rect keys). Every core therefore
runs the identical program on different data slices, perfectly balanced.
Cores return unnormalized partial outputs pv^T [64,1024] stacked with the
softmax row-sums l [1,1024] (computed via a ones-column appended to V in
the PV matmul). The host gather merges partials and normalizes.

On-device dataflow per core (all matmuls float32r, fp32 PSUM accumulate):
  - load qt/qr/kr row-slabs of x naturally, PE-transpose into xT (c on
    partitions),
  - packed projections: [Wq|Wk]^T.T @ xT -> q^T,k^T rows; [Wk|Wv] for the
    rect keys; W^T built on-chip by PE-transposing the tiny weights,
  - scores computed transposed: sT[s,t] = k^T-chunk.T @ q^T so softmax's
    exp is a single ACT pass and PV needs no attention transpose,
  - no max-subtraction (scores ~ N(0,1); exp cannot overflow fp32),
  - causal masking by adding a -1e30 mask (one precomputed [128,896]
    sliding-window tile) to diagonal-crossing score blocks before exp,
  - PV: vAug[s-chunk,65] stationary (col 64 = ones -> row sums for free),
    accumulated over s-chunks in PSUM -> [65, t] partial output.
"""

import numpy as np

import concourse.bacc as bacc
import concourse.mybir as mybir
import concourse.tile as tile
from concourse.bass_utils import run_bass_kernel_spmd
from concourse.masks import make_identity

HEAD = 64
EMB = 1024
T = 2048
BATCH = 4
HALF = 1024  # triangle queries per core
KRECT = 512  # rectangle keys per core
NCHUNK = EMB // 128  # 8 contraction chunks
DT = mybir.dt.float32r  # matmul operand dtype
F32 = mybir.dt.float32
SCALE = 1.0 / 8.0  # 1/sqrt(HEAD)

# xT column layout: [Qt | Qr | Kr]
QT_OFF, QR_OFF, KR_OFF = 0, HALF, 2 * HALF
XCOLS = 2 * HALF + KRECT  # 2560


def build_nc(loop_n=1):
    nc = bacc.Bacc(
        "TRN2", target_bir_lowering=False, debug=False, num_devices=8
    )
    qt_d = nc.dram_tensor("qt", [HALF, EMB], DT, kind="ExternalInput")
    qr_d = nc.dram_tensor("qr", [HALF, EMB], DT, kind="ExternalInput")
    kr_d = nc.dram_tensor("kr", [KRECT, EMB], DT, kind="ExternalInput")
    wq_d = nc.dram_tensor("Wq", [HEAD, EMB], DT, kind="ExternalInput")
    wk_d = nc.dram_tensor("Wk", [HEAD, EMB], DT, kind="ExternalInput")
    wv_d = nc.dram_tensor("Wv", [HEAD, EMB], DT, kind="ExternalInput")
    tri_d = nc.dram_tensor("tri_o", [HEAD + 1, HALF], F32, kind="ExternalOutput")
    rect_d = nc.dram_tensor("rect_o", [HEAD + 1, HALF], F32, kind="ExternalOutput")

    import contextlib

    with tile.TileContext(nc) as tc:
        loop_cm = tc.For_i(0, loop_n, 1) if loop_n > 1 else contextlib.nullcontext()
        with (
            tc.tile_pool(name="const", bufs=1) as cpool,
            tc.tile_pool(name="xnat", bufs=2) as xpool,
            tc.tile_pool(name="xt", bufs=1) as xtpool,
            tc.tile_pool(name="proj", bufs=1) as projpool,
            tc.tile_pool(name="pt", bufs=3) as ptpool,
            tc.tile_pool(name="outp", bufs=2) as outpool,
            tc.tile_pool(name="ps", bufs=1, space="PSUM") as ps,
            loop_cm,
        ):
            # ---- constants ----
            ident_f = cpool.tile([128, 128], F32)
            make_identity(nc, ident_f[:])
            # DVE cast-copy is the fp32r "rounding producer" the verifier wants
            ident = cpool.tile([128, 128], DT)
            nc.vector.tensor_copy(ident[:], ident_f[:])
            # Causal masks in the permuted t-order. Slab loads put x row
            # t = 4p+u on partition p, free slot u, so within a 512-col
            # group the column z = u*128+p holds t(z) = 4*(z%128) + z//128,
            # and an s-chunk i (= group i//4, phase w = i%4) holds
            # s(x) = 4x + w on its partition x. A diagonal block (same
            # group) keeps score [x, z] iff t(z) >= s(x):
            #   4*(z%128) + z//128 - 4x - w >= 0
            # One additive 0/-1e30 mask tile per phase w.
            pmasks = []
            for w in range(4):
                pm = cpool.tile([128, 4, 128], F32, name=f"pmask{w}")
                nc.gpsimd.memset(pm[:], 0.0)
                nc.gpsimd.affine_select(
                    out=pm[:],
                    in_=pm[:],
                    compare_op=mybir.AluOpType.is_ge,
                    fill=-1e30,
                    base=-w,
                    pattern=[[1, 4], [4, 128]],
                    channel_multiplier=-4,
                )
                pmasks.append(pm)

            # ---- weights: load naturally, PE-transpose into packed lhsT ----
            wq_sb = cpool.tile([HEAD, EMB], DT)
            wk_sb = cpool.tile([HEAD, EMB], DT)
            wv_sb = cpool.tile([HEAD, EMB], DT)
            nc.sync.dma_start(wq_sb[:], wq_d[:])
            nc.sync.dma_start(wk_sb[:], wk_d[:])
            nc.sync.dma_start(wv_sb[:], wv_d[:])
            wqk_t = cpool.tile([128, NCHUNK, 128], DT)  # [WqT | WkT] per chunk
            wkv_t = cpool.tile([128, NCHUNK, 128], DT)  # [WkT | WvT] per chunk
            wv_t = cpool.tile([128, NCHUNK, HEAD], DT)
            for j in range(NCHUNK):
                cs = slice(j * 128, (j + 1) * 128)
                for w_sb, dsts in (
                    (wq_sb, [(wqk_t, 0)]),
                    (wk_sb, [(wqk_t, 64), (wkv_t, 0)]),
                    (wv_sb, [(wkv_t, 64), (wv_t, 0)]),
                ):
                    wtp = ps.tile([128, HEAD], DT, tag="bank", name="wtp", bufs=5)
                    nc.tensor.transpose(wtp[:], w_sb[:, cs], ident[0:HEAD, 0:HEAD])
                    for dst, col in dsts:
                        # ACT is idle at program start; keep DVE free
                        nc.scalar.copy(dst[:, j, col : col + HEAD], wtp[:])

            # ---- load x slabs and transpose into xT [128c, NCHUNK, XCOLS] ----
            # Groups of 4 t-tiles: 4 PE transposes land in one psum bank,
            # drained by a single wide copy (alternating DVE/ACT).
            xt = xtpool.tile([128, NCHUNK, XCOLS], DT)
            ng = [0]

            def load_transpose(src, coff, ntile):
                for g in range(0, ntile, 4):
                    # one 2MB DMA per 4-tile group: [512, EMB] -> [128, 4, EMB]
                    xn = xpool.tile([128, 4, EMB], DT, tag="xn", name="xn")
                    dma_eng = nc.sync if ng[0] % 2 == 0 else nc.scalar
                    ng[0] += 1
                    dma_eng.dma_start(
                        xn[:],
                        src[g * 128 : (g + 4) * 128, :].rearrange(
                            "(b a) c -> b a c", b=128
                        ),
                    )
                    for j in range(NCHUNK):
                        tp = ps.tile([128, 512], DT, tag="bank", name="tp", bufs=5)
                        for u in range(4):
                            nc.tensor.transpose(
                                tp[:, u * 128 : (u + 1) * 128],
                                xn[:, u, j * 128 : (j + 1) * 128],
                                ident[:],
                            )
                        tcol = coff + g * 128
                        dst = xt[:, j, tcol : tcol + 512]
                        if j % 3 == 2:
                            nc.scalar.copy(dst, tp[:])
                        else:
                            nc.vector.tensor_copy(dst, tp[:])

            # ---- projections (accumulate over NCHUNK c-chunks) ----
            def proj_stream(w_lhsT, xt_cols, m_rows, sink):
                """sink(nb, psum[m_rows,512]) for each 512-col block of
                w_lhsT.T @ xT[:, :, xt_cols], accumulated over NCHUNK."""
                width = xt_cols.stop - xt_cols.start
                for nb in range(0, width, 512):
                    nw = min(512, width - nb)
                    pp = ps.tile([m_rows, nw], F32, tag="bank", name="pp", bufs=5)
                    for j in range(NCHUNK):
                        nc.tensor.matmul(
                            pp[:],
                            w_lhsT[:, j, :],
                            xt[:, j, xt_cols.start + nb : xt_cols.start + nb + nw],
                            start=(j == 0),
                            stop=(j == NCHUNK - 1),
                        )
                    sink(nb, pp)

            # Qt slab, then its projections (lets tri attention start while
            # the Qr/Kr slabs are still loading)
            load_transpose(qt_d, QT_OFF, 8)
            qkt_sb = projpool.tile([128, HALF], DT)
            proj_stream(wqk_t, slice(QT_OFF, QT_OFF + HALF), 128,
                        lambda nb, pp: nc.vector.tensor_copy(
                            qkt_sb[:, nb : nb + 512], pp[:]))
            qtT = qkt_sb[0:HEAD, :]
            ktT = projpool.tile([HEAD, HALF], DT)
            # partition-moving copy (rows 64:128 -> 0:64) via SBUF->SBUF DMA
            nc.sync.dma_start(ktT[:], qkt_sb[64:128, :])
            # triangle v^T over Qt
            vtT = projpool.tile([HEAD, HALF], F32)
            proj_stream(wv_t, slice(QT_OFF, QT_OFF + HALF), HEAD,
                        lambda nb, pp: nc.vector.tensor_copy(
                            vtT[:, nb : nb + 512], pp[:]))
            # Qr slab + rect q^T (k half of the packed output is unused)
            load_transpose(qr_d, QR_OFF, 8)
            qrT = projpool.tile([HEAD, HALF], DT)
            proj_stream(wqk_t, slice(QR_OFF, QR_OFF + HALF), 128,
                        lambda nb, pp: nc.vector.tensor_copy(
                            qrT[:, nb : nb + 512], pp[0:HEAD, :]))
            # Kr slab + rect k^T,v^T
            load_transpose(kr_d, KR_OFF, 4)
            kvr_sb = projpool.tile([128, KRECT], DT)
            proj_stream(wkv_t, slice(KR_OFF, KR_OFF + KRECT), 128,
                        lambda nb, pp: nc.vector.tensor_copy(
                            kvr_sb[:, nb : nb + 512], pp[:]))
            krT = kvr_sb[0:HEAD, :]
            vrT = projpool.tile([HEAD, KRECT], F32)
            nc.sync.dma_start(vrT[:], kvr_sb[64:128, :].bitcast(F32))

            # ---- vAug chunks: v natural [128,65] with ones column ----
            def make_vaug(vT, nchunks, name):
                tiles = []
                for i in range(nchunks):
                    va = projpool.tile(
                        [128, HEAD + 1], DT, tag=f"{name}{i}", name=f"{name}{i}"
                    )
                    vp = ps.tile([128, HEAD], F32, tag="bank", name="vp", bufs=5)
                    nc.tensor.transpose(
                        vp[:], vT[:, i * 128 : (i + 1) * 128], ident_f[0:HEAD, 0:HEAD]
                    )
                    nc.vector.tensor_copy(va[:, 0:HEAD], vp[:])
                    nc.gpsimd.memset(va[:, HEAD : HEAD + 1].bitcast(F32), 1.0)
                    tiles.append(va)
                return tiles

            vaug_t = make_vaug(vtT, 8, "vat")
            vaug_r = make_vaug(vrT, 4, "var")

            # ---- attention: per t-block of 512, loop s-chunks ----
            def attention(qT, kT, vaug, nschunk, out_d, causal):
                ostage = outpool.tile(
                    [HEAD + 1, HALF], F32, tag="ostage", name="ostage", bufs=2
                )
                for tb in range(HALF // 512):
                    pv = ps.tile([HEAD + 1, 512], F32, tag="pv", name="pv", bufs=2)
                    schunks = [
                        i for i in range(nschunk)
                        if not causal or i // 4 <= tb
                    ]
                    for idx, i in enumerate(schunks):
                        st = ps.tile([128, 512], F32, tag="bank", name="st", bufs=5)
                        nc.tensor.matmul(
                            st[:],
                            kT[:, i * 128 : (i + 1) * 128],
                            qT[:, tb * 512 : (tb + 1) * 512],
                            start=True,
                            stop=True,
                        )
                        if causal and i // 4 == tb:
                            # diagonal block (same 512-group): phase mask
                            nc.vector.tensor_tensor(
                                out=st[:],
                                in0=st[:],
                                in1=pmasks[i % 4][:].rearrange("p a b -> p (a b)"),
                                op=mybir.AluOpType.add,
                            )
                        pt = ptpool.tile([128, 512], DT, tag="pt", name="pt")
                        nc.scalar.activation(
                            pt[:],
                            st[:],
                            mybir.ActivationFunctionType.Exp,
                            bias=0.0,
                            scale=SCALE,
                        )
                        nc.tensor.matmul(
                            pv[:],
                            vaug[i][:],
                            pt[:],
                            start=(idx == 0),
                            stop=(idx == len(schunks) - 1),
                        )
                    nc.vector.tensor_copy(
                        ostage[:, tb * 512 : (tb + 1) * 512], pv[:]
                    )
                nc.sync.dma_start(out_d[:], ostage[:])

            attention(qtT, ktT, vaug_t, 8, tri_d, causal=True)
            attention(qrT, krT, vaug_r, 4, rect_d, causal=False)

    nc.compile()
    return nc


_NC_CACHE = None


def kernel(x, Wq, Wk, Wv):
    global _NC_CACHE
    if _NC_CACHE is None:
        _NC_CACHE = build_nc()
    nc = _NC_CACHE

    x = np.ascontiguousarray(x, dtype=np.float32)
    w = {
        "Wq": np.ascontiguousarray(Wq, dtype=np.float32),
        "Wk": np.ascontiguousarray(Wk, dtype=np.float32),
        "Wv": np.ascontiguousarray(Wv, dtype=np.float32),
    }
    in_maps = []
    for b in range(BATCH):
        for h in range(2):
            in_maps.append(
                {
                    "qt": x[b, h * HALF : (h + 1) * HALF],
                    "qr": x[b, HALF:T],
                    "kr": x[b, h * KRECT : (h + 1) * KRECT],
                    **w,
                }
            )
    res = run_bass_kernel_spmd(nc, in_maps, core_ids=list(range(8)))

    # un-permute device column order: col z of each 512-group holds
    # t = 4*(z%128) + z//128 (see the slab-load layout in build_nc)
    z = np.arange(512)
    t_of_z = 4 * (z % 128) + z // 128
    perm = np.concatenate([512 * g + t_of_z for g in range(HALF // 512)])

    def unperm(a):
        r = np.empty_like(a)
        r[:, perm] = a
        return r

    out = np.empty((BATCH, T, HEAD), dtype=np.float32)
    for b in range(BATCH):
        r0 = {k: unperm(v) for k, v in res.results[2 * b].items()}
        r1 = {k: unperm(v) for k, v in res.results[2 * b + 1].items()}
        t0 = r0["tri_o"].astype(np.float64)
        out[b, 0:HALF] = (t0[0:HEAD] / t0[HEAD]).T
        num = (
            r1["tri_o"][0:HEAD].astype(np.float64)
            + r0["rect_o"][0:HEAD].astype(np.float64)
            + r1["rect_o"][0:HEAD].astype(np.float64)
        )
        den = (
            r1["tri_o"][HEAD].astype(np.float64)
            + r0["rect_o"][HEAD].astype(np.float64)
            + r1["rect_o"][HEAD].astype(np.float64)
        )
        out[b, HALF:T] = (num / den).T
    return out
